# revision 3
# baseline (speedup 1.0000x reference)
"""AttCML distributed Bass kernel for 8 TRN2 NeuronCores — exact-packing v2.

Data-parallel over batch; both attention contractions on the PE array.

Key design points (v2, vs. the geometry-pool baseline):

  - input-adaptive packing: the Bass program is built inside kernel() after
    seeing n_prefs.  Samples are sorted by v = n_prefs+1 descending and dealt
    round-robin to the 8 cores by rank, so all cores share one program whose
    per-rank segment size is the max v across cores at that rank (+0.04%).
    Segments are first-fit-decreasing bin-packed into 128-slot "quads"
    (26.1 slots/sample vs 37.2 for fixed pools), then quads are FFD-packed by
    m = 2*samples into 128-column PSUM groups; 4 groups = one superblock
    ([128,512] = one PSUM bank).
  - per-column additive mask tensor [128, NCOL] shipped from host replaces
    the per-geometry block masks AND the pad-count correction: rows beyond a
    sample's v get -30 so fp8 exp underflows to exact 0.
  - all DMA-resident tiles are distinct SBUF allocations (no buffer reuse →
    no mid-stream DMA stalls); transfers are issued 4 superblocks ahead on
    the two HWDGE rings (sync: tgt+prefT+mask, scalar: prefQ+d0).
  - softmax denominator: ones-matmul broadcast of S + dead-col +1 via a
    [1,NCOL] row matmul; 1/S via reciprocal_approx_fast (custom DVE, ~5x
    faster than InstReciprocal and more accurate than the old bf16 path).
  - distances: rm = rps*(1/S) (DVE), qv = rm + d0 (DVE), square (ACT),
    ones-column matmul over d, [1,512] row copied and DMAed per superblock
    on the sync ring (queued behind the remaining input stream, so it never
    delays the tail).

Pad slots inside a segment are masked (-30); zero-id slots below v are real
zero rows contributing exp(0)=1 — exact reference semantics.
"""

import numpy as np
from contextlib import ExitStack

try:
    import concourse  # noqa: F401
except ImportError:  # pragma: no cover
    import sys

    for _p in ("/opt/trn_rl_repo", "/root/.axon_site/_ro/trn_rl_repo"):
        if _p not in sys.path:
            sys.path.insert(0, _p)

import ml_dtypes
import concourse.bacc as bacc
import concourse.bass as bass
import concourse.tile as tile
from concourse import mybir
from concourse.bass_utils import run_bass_kernel_spmd

F32 = mybir.dt.float32
BF16 = mybir.dt.bfloat16
FP8 = mybir.dt.float8e3  # e3m4
ALU = mybir.AluOpType
ACTF = mybir.ActivationFunctionType

FP8NP = ml_dtypes.float8_e3m4
BF16NP = ml_dtypes.bfloat16

D = 128
P = 50
N_CORES = 8
B = 16384
NRANK = B // N_CORES
MASKVAL = -30.0
LOOK = 4  # superblock DMA lookahead


class Packing:
    """Compile-time packing derived from the (global) v = n_prefs+1 array."""

    def __init__(self, v):
        order = np.argsort(-v, kind="stable")
        vs = v[order].reshape(NRANK, N_CORES)
        seg = vs.max(axis=1).astype(np.int64)  # descending

        # FFD: ranks into 128-slot quads
        quads = []  # [remaining, [rank, ...]]
        for i in range(NRANK):
            s = int(seg[i])
            for q in quads:
                if q[0] >= s:
                    q[0] -= s
                    q[1].append(i)
                    break
            else:
                quads.append([128 - s, [i]])

        # FFD: quads into 128-col groups by m = 2*samples
        ms = sorted(((2 * len(q[1]), qi) for qi, q in enumerate(quads)),
                    reverse=True)
        groups = []  # [remaining_cols, [quad_id, ...]]
        for m, qi in ms:
            for g in groups:
                if g[0] >= m:
                    g[0] -= m
                    g[1].append(qi)
                    break
            else:
                groups.append([128 - m, [qi]])

        NG = len(groups)
        # renumber quads in group order -> contiguous slot array per group
        self.order = order
        self.seg = seg
        self.NG = NG
        self.NCOL = NG * 128
        # per group: list of (new_quad_idx, colbase, [(rank, rowbase, seg)])
        self.groups = []
        rank_info = np.zeros((NRANK, 3), np.int64)  # quad, rowbase, colbase
        qn = 0
        for g, (_rem, qids) in enumerate(groups):
            glist = []
            col = g * 128
            for qi in qids:
                rows = []
                rb = 0
                for k, r in enumerate(quads[qi][1]):
                    rank_info[r] = (qn, rb, col + 2 * k)
                    rows.append((r, rb, int(seg[r])))
                    rb += int(seg[r])
                glist.append((qn, col, 2 * len(rows), rows))
                col += 2 * len(rows)
                qn += 1
            self.groups.append(glist)
        self.NQ = qn
        self.rank_info = rank_info

        # superblocks: small ramp-in and ramp-out, 4-group body
        sizes = [1, 1, 2]
        rem = NG - 4 - 4  # reserve [2,1,1] tail
        while rem >= 4:
            sizes.append(4)
            rem -= 4
        if rem:
            sizes.append(rem)
        sizes += [2, 1, 1]
        assert sum(sizes) == NG
        self.SBS = []
        g0 = 0
        for sz in sizes:
            self.SBS.append((g0, sz))
            g0 += sz

        # per-sb quad ranges (quads are contiguous per group, groups per sb)
        self.sb_q = []
        for g0, ng in self.SBS:
            q0 = self.groups[g0][0][0]
            qlast = self.groups[g0 + ng - 1][-1]
            self.sb_q.append((q0, qlast[0] + 1))

        # vectorization helpers for host prep
        lens = seg  # per rank
        self.tot = int(lens.sum())
        starts = rank_info[:, 0] * 128 + rank_info[:, 1]  # flat slot index
        rep_rank = np.repeat(np.arange(NRANK), lens)
        off_in_seg = np.arange(self.tot) - np.repeat(
            np.cumsum(lens) - lens, lens
        )
        self.flat_slot = np.repeat(starts, lens) + off_in_seg
        self.rep_rank = rep_rank
        self.off_in_seg = off_in_seg


def build_bass(pk: Packing):
    NQ, NCOL = pk.NQ, pk.NCOL
    SBS = pk.SBS
    NSB = len(SBS)

    nc = bacc.Bacc(
        "TRN2",
        target_bir_lowering=False,
        debug=False,
        enable_asserts=False,
        num_devices=N_CORES,
    )

    prefT_in = nc.declare_dram_parameter("prefT", [128, NQ * 128], FP8, isOutput=False)
    prefQ_in = nc.declare_dram_parameter("prefQ", [128, NQ * 128], FP8, isOutput=False)
    tgt_in = nc.declare_dram_parameter("tgt", [128, NCOL], FP8, isOutput=False)
    d0_in = nc.declare_dram_parameter("d0", [128, NCOL], BF16, isOutput=False)
    maskb_in = nc.declare_dram_parameter("maskb", [128, NCOL], BF16, isOutput=False)
    padc_in = nc.declare_dram_parameter("padc", [1, NCOL], BF16, isOutput=False)
    ones8_in = nc.declare_dram_parameter("ones8", [128, 128], FP8, isOutput=False)
    onesr_in = nc.declare_dram_parameter("onesr", [1, 128], BF16, isOutput=False)
    onesc_in = nc.declare_dram_parameter("onesc", [128, 1], BF16, isOutput=False)
    out_d = nc.declare_dram_parameter("out", [1, NCOL], F32, isOutput=True)

    with tile.TileContext(nc) as tc, ExitStack() as ctx:
        ctx.enter_context(
            nc.allow_low_precision(reason="fp8/bf16 pipeline validated vs reference")
        )
        consts = ctx.enter_context(tc.tile_pool(name="consts", bufs=1))
        res_pool = ctx.enter_context(tc.tile_pool(name="res", bufs=1))
        sm_pool = ctx.enter_context(tc.tile_pool(name="sm", bufs=3))
        row_pool = ctx.enter_context(tc.tile_pool(name="row", bufs=2))
        w_ps = ctx.enter_context(
            tc.tile_pool(name="wps", bufs=2, space=bass.MemorySpace.PSUM)
        )
        s_ps = ctx.enter_context(
            tc.tile_pool(name="sps", bufs=2, space=bass.MemorySpace.PSUM)
        )
        r_ps = ctx.enter_context(
            tc.tile_pool(name="rps", bufs=2, space=bass.MemorySpace.PSUM)
        )
        o_ps = ctx.enter_context(
            tc.tile_pool(name="ops", bufs=2, space=bass.MemorySpace.PSUM)
        )

        # consts on the scalar ring (small, needed from the first finish())
        ones8 = consts.tile([128, 128], FP8)
        nc.scalar.dma_start(ones8[:], ones8_in[:])
        onesr = consts.tile([1, 128], BF16)
        nc.scalar.dma_start(onesr[:], onesr_in[:])
        onesc = consts.tile([128, 1], BF16)
        nc.scalar.dma_start(onesc[:], onesc_in[:])
        padcr = consts.tile([1, NCOL], BF16)
        nc.scalar.dma_start(padcr[:], padc_in[:])

        sbT = [None] * NSB
        sbC = [None] * NSB

        def issue_T(sb):
            g0, ng = SBS[sb]
            q0, q1 = pk.sb_q[sb]
            wid = ng * 128
            tg = res_pool.tile([128, wid], FP8, tag=f"tg{sb}", name="tg")
            nc.sync.dma_start(tg[:], tgt_in[:, g0 * 128 : g0 * 128 + wid])
            pT = res_pool.tile([128, (q1 - q0) * 128], FP8, tag=f"pT{sb}", name="pT")
            nc.sync.dma_start(pT[:], prefT_in[:, q0 * 128 : q1 * 128])
            mk = res_pool.tile([128, wid], BF16, tag=f"mk{sb}", name="mk")
            nc.sync.dma_start(mk[:], maskb_in[:, g0 * 128 : g0 * 128 + wid])
            sbT[sb] = (tg, pT, mk)

        def issue_C(sb):
            g0, ng = SBS[sb]
            q0, q1 = pk.sb_q[sb]
            wid = ng * 128
            pQ = res_pool.tile([128, (q1 - q0) * 128], FP8, tag=f"pQ{sb}", name="pQ")
            nc.scalar.dma_start(pQ[:], prefQ_in[:, q0 * 128 : q1 * 128])
            d0 = res_pool.tile([128, wid], BF16, tag=f"d0{sb}", name="d0")
            nc.scalar.dma_start(d0[:], d0_in[:, g0 * 128 : g0 * 128 + wid])
            sbC[sb] = (pQ, d0)

        def stage_a(sb):
            g0, ng = SBS[sb]
            tg, pT, mk = sbT[sb]
            q0, _ = pk.sb_q[sb]
            c0 = g0 * 128
            wps = w_ps.tile([128, 512], F32, tag="w", name="wps")
            for g in range(g0, g0 + ng):
                for qn, col, m, _rows in pk.groups[g]:
                    lc = col - c0
                    nc.tensor.matmul(
                        wps[:, lc : lc + m],
                        pT[:, (qn - q0) * 128 : (qn - q0 + 1) * 128],
                        tg[:, lc : lc + m],
                    )
            return wps

        def finish(sb, wps):
            g0, ng = SBS[sb]
            tg, pT, mk = sbT[sb]
            pQ, d0 = sbC[sb]
            q0, _ = pk.sb_q[sb]
            c0 = g0 * 128
            wid = ng * 128

            wm = sm_pool.tile([128, 512], BF16, tag="wm", name="wm")
            nc.vector.tensor_tensor(
                out=wm[:, :wid], in0=wps[:, :wid], in1=mk[:, :wid], op=ALU.add
            )
            ee = sm_pool.tile([128, 512], FP8, tag="ee", name="ee")
            nc.scalar.activation(ee[:, :wid], wm[:, :wid], ACTF.Exp)

            sps = s_ps.tile([128, 512], F32, tag="s", name="sps")
            nc.tensor.matmul(
                sps[:, :wid], ones8[:], ee[:, :wid], start=True, stop=False
            )
            nc.tensor.matmul(
                sps[:, :wid],
                onesr[:],
                padcr[:, c0 : c0 + wid],
                start=False,
                stop=True,
                skip_group_check=True,
            )
            rs = sm_pool.tile([128, 512], F32, tag="rs", name="rs")
            nc.vector.reciprocal_approx_fast(rs[:, :wid], sps[:, :wid])

            rps = r_ps.tile([128, 512], F32, tag="r", name="rps")
            for g in range(g0, g0 + ng):
                for qn, col, m, _rows in pk.groups[g]:
                    lc = col - c0
                    nc.tensor.matmul(
                        rps[:, lc : lc + m],
                        pQ[:, (qn - q0) * 128 : (qn - q0 + 1) * 128],
                        ee[:, lc : lc + m],
                    )

            rm = sm_pool.tile([128, 512], BF16, tag="rm", name="rm")
            nc.vector.tensor_tensor(
                out=rm[:, :wid], in0=rps[:, :wid], in1=rs[:, :wid], op=ALU.mult
            )
            qv = sm_pool.tile([128, 512], BF16, tag="qv", name="qv")
            nc.vector.tensor_tensor(
                out=qv[:, :wid], in0=rm[:, :wid], in1=d0[:, :wid], op=ALU.add
            )
            q2 = sm_pool.tile([128, 512], BF16, tag="q2", name="q2")
            nc.gpsimd.tensor_mul(q2[:, :wid], qv[:, :wid], qv[:, :wid])
            pend[0] = (q2, c0, wid)

        pend = [None]

        def emit_dist():
            # one iteration behind finish() so the PE never waits on q2
            if pend[0] is None:
                return
            q2, c0, wid = pend[0]
            pend[0] = None
            ops = o_ps.tile([1, 512], F32, tag="o", name="ops")
            nc.tensor.matmul(ops[:, :wid], onesc[:], q2[:, :wid])
            orow = row_pool.tile([1, 512], F32, tag="or", name="orow")
            nc.scalar.copy(orow[:, :wid], ops[:, :wid])
            nc.gpsimd.dma_start(out_d[:, c0 : c0 + wid], orow[:, :wid])

        for sb in range(min(LOOK, NSB)):
            issue_T(sb)
            issue_C(sb)
        wcur = stage_a(0)
        for sb in range(NSB):
            if sb + LOOK < NSB:
                issue_T(sb + LOOK)
                issue_C(sb + LOOK)
            wnext = stage_a(sb + 1) if sb + 1 < NSB else None
            emit_dist()
            finish(sb, wcur)
            wcur = wnext
        emit_dist()

    nc.compile()
    return nc


_CACHE = {}


def _get(v):
    key = v.tobytes()
    if _CACHE.get("key") != key:
        pk = Packing(v)
        nc = build_bass(pk)
        _CACHE.update(key=key, pk=pk, nc=nc)
    return _CACHE["pk"], _CACHE["nc"]


def prep_core(pk, core, ctx32, ctx8, user_emb, user_ids, pos_ids, neg_ids,
              pref_ids, v):
    """Build one core's input map + unscramble info."""
    NQ, NCOL = pk.NQ, pk.NCOL
    ZERO = ctx8.shape[0] - 1

    samples = pk.order[np.arange(NRANK) * N_CORES + core]  # per rank
    vc = v[samples]  # <= seg per construction

    # slot id array [NQ*128]
    sid = np.full(NQ * 128, ZERO, np.int64)
    keep = pk.off_in_seg < vc[pk.rep_rank]
    fs = pk.flat_slot[keep]
    sid[fs] = pref_ids[samples[pk.rep_rank[keep]], pk.off_in_seg[keep]]
    sid = sid.reshape(NQ, 128)

    # mask [128, NCOL]: 0 for (slot rows < vc) of each sample's two columns
    mask = np.full((128, NCOL), MASKVAL, np.float32)
    rows = (pk.rank_info[pk.rep_rank[keep], 1] + pk.off_in_seg[keep])
    colp = pk.rank_info[pk.rep_rank[keep], 2]
    mask[rows, colp] = 0.0
    mask[rows, colp + 1] = 0.0

    # per-column targets
    colsamp = np.full(NCOL, -1, np.int64)
    colt = np.zeros(NCOL, np.int64)
    tid = np.full(NCOL, ZERO, np.int64)
    uid = np.zeros(NCOL, np.int64)
    valid = np.zeros(NCOL, bool)
    cp = pk.rank_info[:, 2]
    for t, t_ids in ((0, pos_ids), (1, neg_ids)):
        cc = cp + t
        colsamp[cc] = samples
        colt[cc] = t
        tid[cc] = t_ids[samples]
        uid[cc] = user_ids[samples]
        valid[cc] = True
    padc = (~valid).astype(np.float32)  # dead cols: S = 0 + 1

    g8 = ctx8[sid]  # [NQ, 128, 128]
    prefQ = np.ascontiguousarray(g8.transpose(1, 0, 2)).reshape(128, NQ * 128)
    prefT = np.ascontiguousarray(g8.transpose(2, 0, 1)).reshape(128, NQ * 128)

    tgt = np.ascontiguousarray(ctx8[tid].T)  # [128, NCOL] fp8
    d0f = user_emb[uid] - ctx32[tid]  # [NCOL, 128] f32
    d0f[~valid] = 0.0
    d0T = np.ascontiguousarray(d0f.T).astype(BF16NP)

    in_map = {
        "prefT": prefT,
        "prefQ": prefQ,
        "tgt": tgt,
        "d0": d0T,
        "maskb": np.ascontiguousarray(mask.astype(BF16NP)),
        "padc": padc.astype(BF16NP).reshape(1, NCOL),
        "ones8": np.ones((128, 128), FP8NP),
        "onesr": np.ones((1, 128), BF16NP),
        "onesc": np.ones((128, 1), BF16NP),
    }
    return in_map, colsamp, colt, valid


def kernel(user_emb, item_emb, user_ids, pos_ids, neg_ids, pref_ids, n_prefs,
           _trace=False):
    user_emb = np.ascontiguousarray(np.asarray(user_emb, np.float32))
    item_emb = np.asarray(item_emb, np.float32)
    user_ids = np.asarray(user_ids).astype(np.int64)
    pos_ids = np.asarray(pos_ids).astype(np.int64)
    neg_ids = np.asarray(neg_ids).astype(np.int64)
    pref_ids = np.asarray(pref_ids).astype(np.int64)
    n_prefs = np.asarray(n_prefs, np.float32)

    ctx32 = np.concatenate([item_emb, np.zeros((1, D), np.float32)], axis=0)
    ctx8 = ctx32.astype(FP8NP)

    v = n_prefs.astype(np.int64) + 1  # valid slot counts

    pk, nc = _get(v)

    in_maps = []
    unscr = []
    for core in range(N_CORES):
        im, colsamp, colt, valid = prep_core(
            pk, core, ctx32, ctx8, user_emb, user_ids, pos_ids, neg_ids,
            pref_ids, v
        )
        in_maps.append(im)
        unscr.append((colsamp, colt, valid))

    res = run_bass_kernel_spmd(
        nc, in_maps, core_ids=list(range(N_CORES)), trace=_trace
    )

    out = np.empty((2, B), dtype=np.float32)
    for core in range(N_CORES):
        r = np.asarray(res.results[core]["out"]).reshape(pk.NCOL)
        colsamp, colt, valid = unscr[core]
        out[colt[valid], colsamp[valid]] = r[valid]
    if _trace:
        return out, res
    return out


# revision 9
# speedup vs baseline: 1.1228x; 1.1228x over previous
"""AttCML distributed Bass kernel for 8 TRN2 NeuronCores — exact-packing v2.

Data-parallel over batch; both attention contractions on the PE array.

Key design points (v2, vs. the geometry-pool baseline):

  - input-adaptive packing: the Bass program is built inside kernel() after
    seeing n_prefs.  Samples are sorted by v = n_prefs+1 descending and dealt
    round-robin to the 8 cores by rank, so all cores share one program whose
    per-rank segment size is the max v across cores at that rank (+0.04%).
    Segments are first-fit-decreasing bin-packed into 128-slot "quads"
    (26.1 slots/sample vs 37.2 for fixed pools), then quads are FFD-packed by
    m = 2*samples into 128-column PSUM groups; 4 groups = one superblock
    ([128,512] = one PSUM bank).
  - per-column additive mask tensor [128, NCOL] shipped from host replaces
    the per-geometry block masks AND the pad-count correction: rows beyond a
    sample's v get -30 so fp8 exp underflows to exact 0.
  - all DMA-resident tiles are distinct SBUF allocations (no buffer reuse →
    no mid-stream DMA stalls); transfers are issued 4 superblocks ahead on
    the two HWDGE rings (sync: tgt+prefT+mask, scalar: prefQ+d0).
  - softmax denominator: ones-matmul broadcast of S + dead-col +1 via a
    [1,NCOL] row matmul; 1/S via reciprocal_approx_fast (custom DVE, ~5x
    faster than InstReciprocal and more accurate than the old bf16 path).
  - distances: rm = rps*(1/S) (DVE), qv = rm + d0 (DVE), square (ACT),
    ones-column matmul over d, [1,512] row copied and DMAed per superblock
    on the sync ring (queued behind the remaining input stream, so it never
    delays the tail).

Pad slots inside a segment are masked (-30); zero-id slots below v are real
zero rows contributing exp(0)=1 — exact reference semantics.
"""

import numpy as np
from contextlib import ExitStack

try:
    import concourse  # noqa: F401
except ImportError:  # pragma: no cover
    import sys

    for _p in ("/opt/trn_rl_repo", "/root/.axon_site/_ro/trn_rl_repo"):
        if _p not in sys.path:
            sys.path.insert(0, _p)

import ml_dtypes
import concourse.bacc as bacc
import concourse.bass as bass
import concourse.tile as tile
from concourse import mybir
from concourse.bass_utils import run_bass_kernel_spmd

F32 = mybir.dt.float32
BF16 = mybir.dt.bfloat16
FP8 = mybir.dt.float8e3  # e3m4
ALU = mybir.AluOpType
ACTF = mybir.ActivationFunctionType

FP8NP = ml_dtypes.float8_e3m4
BF16NP = ml_dtypes.bfloat16

D = 128
P = 50
N_CORES = 8
B = 16384
NRANK = B // N_CORES
MASKVAL = -30.0
LOOK = 4  # superblock DMA lookahead


class Packing:
    """Compile-time packing derived from the (global) v = n_prefs+1 array."""

    def __init__(self, v):
        order = np.argsort(-v, kind="stable")
        vs = v[order].reshape(NRANK, N_CORES)
        seg = vs.max(axis=1).astype(np.int64)  # descending

        # FFD: ranks into 128-slot quads
        quads = []  # [remaining, [rank, ...]]
        for i in range(NRANK):
            s = int(seg[i])
            for q in quads:
                if q[0] >= s:
                    q[0] -= s
                    q[1].append(i)
                    break
            else:
                quads.append([128 - s, [i]])

        # FFD: quads into 128-col groups by m = 2*samples
        ms = sorted(((2 * len(q[1]), qi) for qi, q in enumerate(quads)),
                    reverse=True)
        groups = []  # [remaining_cols, [quad_id, ...]]
        for m, qi in ms:
            for g in groups:
                if g[0] >= m:
                    g[0] -= m
                    g[1].append(qi)
                    break
            else:
                groups.append([128 - m, [qi]])

        NG = len(groups)
        # renumber quads in group order -> contiguous slot array per group
        self.order = order
        self.seg = seg
        self.NG = NG
        self.NCOL = NG * 128
        # per group: list of (new_quad_idx, colbase, [(rank, rowbase, seg)])
        self.groups = []
        rank_info = np.zeros((NRANK, 3), np.int64)  # quad, rowbase, colbase
        qn = 0
        for g, (_rem, qids) in enumerate(groups):
            glist = []
            col = g * 128
            for qi in qids:
                rows = []
                rb = 0
                for k, r in enumerate(quads[qi][1]):
                    rank_info[r] = (qn, rb, col + 2 * k)
                    rows.append((r, rb, int(seg[r])))
                    rb += int(seg[r])
                glist.append((qn, col, 2 * len(rows), rows))
                col += 2 * len(rows)
                qn += 1
            self.groups.append(glist)
        self.NQ = qn
        self.rank_info = rank_info

        # superblocks: small ramp-in and ramp-out, 4-group body
        sizes = [1, 1, 2]
        rem = NG - 4 - 4  # reserve [2,1,1] tail
        while rem >= 4:
            sizes.append(4)
            rem -= 4
        if rem:
            sizes.append(rem)
        sizes += [2, 1, 1]
        assert sum(sizes) == NG
        self.SBS = []
        g0 = 0
        for sz in sizes:
            self.SBS.append((g0, sz))
            g0 += sz

        # per-sb quad ranges (quads are contiguous per group, groups per sb)
        self.sb_q = []
        for g0, ng in self.SBS:
            q0 = self.groups[g0][0][0]
            qlast = self.groups[g0 + ng - 1][-1]
            self.sb_q.append((q0, qlast[0] + 1))

        # vectorization helpers for host prep
        lens = seg  # per rank
        self.tot = int(lens.sum())
        starts = rank_info[:, 0] * 128 + rank_info[:, 1]  # flat slot index
        rep_rank = np.repeat(np.arange(NRANK), lens)
        off_in_seg = np.arange(self.tot) - np.repeat(
            np.cumsum(lens) - lens, lens
        )
        self.flat_slot = np.repeat(starts, lens) + off_in_seg
        self.rep_rank = rep_rank
        self.off_in_seg = off_in_seg


def build_bass(pk: Packing):
    NQ, NCOL = pk.NQ, pk.NCOL
    SBS = pk.SBS
    NSB = len(SBS)

    nc = bacc.Bacc(
        "TRN2",
        target_bir_lowering=False,
        debug=False,
        enable_asserts=False,
        num_devices=N_CORES,
    )

    prefT_in = nc.declare_dram_parameter("prefT", [128, NQ * 128], FP8, isOutput=False)
    prefQ_in = nc.declare_dram_parameter("prefQ", [128, NQ * 128], FP8, isOutput=False)
    tgt_in = nc.declare_dram_parameter("tgt", [128, NCOL], FP8, isOutput=False)
    d0_in = nc.declare_dram_parameter("d0", [128, NCOL], BF16, isOutput=False)
    maskb_in = nc.declare_dram_parameter("maskb", [128, NCOL], BF16, isOutput=False)
    ones8_in = nc.declare_dram_parameter("ones8", [128, 128], FP8, isOutput=False)
    onesc_in = nc.declare_dram_parameter("onesc", [128, 1], BF16, isOutput=False)
    out_d = nc.declare_dram_parameter("out", [1, NCOL], F32, isOutput=True)
    s_d = nc.declare_dram_parameter("sout", [1, NCOL], F32, isOutput=True)

    with tile.TileContext(nc) as tc, ExitStack() as ctx:
        ctx.enter_context(
            nc.allow_low_precision(reason="fp8/bf16 pipeline validated vs reference")
        )
        consts = ctx.enter_context(tc.tile_pool(name="consts", bufs=1))
        res_pool = ctx.enter_context(tc.tile_pool(name="res", bufs=1))
        sm_pool = ctx.enter_context(tc.tile_pool(name="sm", bufs=3))
        row_pool = ctx.enter_context(tc.tile_pool(name="row", bufs=2))
        w_ps = ctx.enter_context(
            tc.tile_pool(name="wps", bufs=2, space=bass.MemorySpace.PSUM)
        )
        s_ps = ctx.enter_context(
            tc.tile_pool(name="sps", bufs=2, space=bass.MemorySpace.PSUM)
        )
        r_ps = ctx.enter_context(
            tc.tile_pool(name="rps", bufs=2, space=bass.MemorySpace.PSUM)
        )
        o_ps = ctx.enter_context(
            tc.tile_pool(name="ops", bufs=2, space=bass.MemorySpace.PSUM)
        )

        # consts on the scalar ring (small, needed from the first superblock)
        ones8 = consts.tile([128, 128], FP8)
        nc.scalar.dma_start(ones8[:], ones8_in[:])
        onesc = consts.tile([128, 1], BF16)
        nc.scalar.dma_start(onesc[:], onesc_in[:])

        sbT = [None] * NSB
        sbC = [None] * NSB

        def issue_T(sb):
            g0, ng = SBS[sb]
            q0, q1 = pk.sb_q[sb]
            wid = ng * 128
            tg = res_pool.tile([128, wid], FP8, tag=f"tg{sb}", name="tg")
            nc.sync.dma_start(tg[:], tgt_in[:, g0 * 128 : g0 * 128 + wid])
            pT = res_pool.tile([128, (q1 - q0) * 128], FP8, tag=f"pT{sb}", name="pT")
            nc.sync.dma_start(pT[:], prefT_in[:, q0 * 128 : q1 * 128])
            mk = res_pool.tile([128, wid], BF16, tag=f"mk{sb}", name="mk")
            nc.sync.dma_start(mk[:], maskb_in[:, g0 * 128 : g0 * 128 + wid])
            sbT[sb] = (tg, pT, mk)

        def issue_C(sb):
            g0, ng = SBS[sb]
            q0, q1 = pk.sb_q[sb]
            wid = ng * 128
            pQ = res_pool.tile([128, (q1 - q0) * 128], FP8, tag=f"pQ{sb}", name="pQ")
            nc.scalar.dma_start(pQ[:], prefQ_in[:, q0 * 128 : q1 * 128])
            d0 = res_pool.tile([128, wid], BF16, tag=f"d0{sb}", name="d0")
            nc.scalar.dma_start(d0[:], d0_in[:, g0 * 128 : g0 * 128 + wid])
            sbC[sb] = (pQ, d0)

        def quads_mm(sb, dst, src_pref, mov):
            g0, ng = SBS[sb]
            q0, _ = pk.sb_q[sb]
            c0 = g0 * 128
            for g in range(g0, g0 + ng):
                for qn, col, m, _rows in pk.groups[g]:
                    lc = col - c0
                    nc.tensor.matmul(
                        dst[:, lc : lc + m],
                        src_pref[:, (qn - q0) * 128 : (qn - q0 + 1) * 128],
                        mov[:, lc : lc + m],
                    )

        pend = [None]

        def emit_dist():
            # one iteration behind, so the PE/ACT never wait on fresh q2
            if pend[0] is None:
                return
            q2, c0, wid = pend[0]
            pend[0] = None
            ops = o_ps.tile([1, 512], F32, tag="o", name="ops")
            nc.tensor.matmul(ops[:, :wid], onesc[:], q2[:, :wid])
            orow = row_pool.tile([1, 512], F32, tag="or", name="orow")
            nc.scalar.copy(orow[:, :wid], ops[:, :wid])
            nc.gpsimd.dma_start(out_d[:, c0 : c0 + wid], orow[:, :wid])

        def stage_a(sb):
            tg, pT, mk = sbT[sb]
            wps = w_ps.tile([128, 512], F32, tag="w", name="wps")
            quads_mm(sb, wps, pT, tg)
            return wps

        def finish(sb, wps):
            g0, ng = SBS[sb]
            tg, pT, mk = sbT[sb]
            pQ, d0 = sbC[sb]
            c0 = g0 * 128
            wid = ng * 128

            wm = sm_pool.tile([128, 512], BF16, tag="wm", name="wm")
            nc.vector.tensor_tensor(
                out=wm[:, :wid], in0=wps[:, :wid], in1=mk[:, :wid], op=ALU.add
            )
            ee = sm_pool.tile([128, 512], FP8, tag="ee", name="ee")
            nc.scalar.activation(ee[:, :wid], wm[:, :wid], ACTF.Exp)

            # S broadcast to all partitions; row 0 exported for the host divide
            sps = s_ps.tile([128, 512], F32, tag="s", name="sps")
            nc.tensor.matmul(sps[:, :wid], ones8[:], ee[:, :wid])
            srow = row_pool.tile([1, 512], F32, tag="sr", name="srow")
            nc.vector.tensor_copy(out=srow[:, :wid], in_=sps[0:1, :wid])
            nc.gpsimd.dma_start(s_d[:, c0 : c0 + wid], srow[:, :wid])

            rps = r_ps.tile([128, 512], F32, tag="r", name="rps")
            quads_mm(sb, rps, pQ, ee)

            # qv = S*d0 + rps  (distances are divided by S^2 on the host)
            m1 = sm_pool.tile([128, 512], BF16, tag="m1", name="m1")
            nc.vector.tensor_tensor(
                out=m1[:, :wid], in0=sps[:, :wid], in1=d0[:, :wid], op=ALU.mult
            )
            qv = sm_pool.tile([128, 512], BF16, tag="qv", name="qv")
            nc.vector.tensor_tensor(
                out=qv[:, :wid], in0=rps[:, :wid], in1=m1[:, :wid], op=ALU.add
            )
            q2 = sm_pool.tile([128, 512], BF16, tag="q2", name="q2")
            nc.vector.tensor_tensor(
                out=q2[:, :wid], in0=qv[:, :wid], in1=qv[:, :wid], op=ALU.mult
            )
            pend[0] = (q2, c0, wid)

        LOOK_T, LOOK_C = 4, 3
        for sb in range(min(LOOK_T, NSB)):
            issue_T(sb)
            if sb < LOOK_C:
                issue_C(sb)
        wcur = stage_a(0)
        for sb in range(NSB):
            if sb + LOOK_T < NSB:
                issue_T(sb + LOOK_T)
            if sb + LOOK_C < NSB:
                issue_C(sb + LOOK_C)
            emit_dist()
            wnext = stage_a(sb + 1) if sb + 1 < NSB else None
            finish(sb, wcur)
            wcur = wnext
        emit_dist()

    nc.compile()
    return nc


_CACHE = {}


def _get(v):
    key = v.tobytes()
    if _CACHE.get("key") != key:
        pk = Packing(v)
        nc = build_bass(pk)
        _CACHE.update(key=key, pk=pk, nc=nc)
    return _CACHE["pk"], _CACHE["nc"]


def prep_core(pk, core, ctx32, ctx8, user_emb, user_ids, pos_ids, neg_ids,
              pref_ids, v):
    """Build one core's input map + unscramble info."""
    NQ, NCOL = pk.NQ, pk.NCOL
    ZERO = ctx8.shape[0] - 1

    samples = pk.order[np.arange(NRANK) * N_CORES + core]  # per rank
    vc = v[samples]  # <= seg per construction

    # slot id array [NQ*128]
    sid = np.full(NQ * 128, ZERO, np.int64)
    keep = pk.off_in_seg < vc[pk.rep_rank]
    fs = pk.flat_slot[keep]
    sid[fs] = pref_ids[samples[pk.rep_rank[keep]], pk.off_in_seg[keep]]
    sid = sid.reshape(NQ, 128)

    # mask [128, NCOL]: 0 for (slot rows < vc) of each sample's two columns
    mask = np.full((128, NCOL), MASKVAL, np.float32)
    rows = (pk.rank_info[pk.rep_rank[keep], 1] + pk.off_in_seg[keep])
    colp = pk.rank_info[pk.rep_rank[keep], 2]
    mask[rows, colp] = 0.0
    mask[rows, colp + 1] = 0.0

    # per-column targets
    colsamp = np.full(NCOL, -1, np.int64)
    colt = np.zeros(NCOL, np.int64)
    tid = np.full(NCOL, ZERO, np.int64)
    uid = np.zeros(NCOL, np.int64)
    valid = np.zeros(NCOL, bool)
    cp = pk.rank_info[:, 2]
    for t, t_ids in ((0, pos_ids), (1, neg_ids)):
        cc = cp + t
        colsamp[cc] = samples
        colt[cc] = t
        tid[cc] = t_ids[samples]
        uid[cc] = user_ids[samples]
        valid[cc] = True

    g8 = ctx8[sid]  # [NQ, 128, 128]
    prefQ = np.ascontiguousarray(g8.transpose(1, 0, 2)).reshape(128, NQ * 128)
    prefT = np.ascontiguousarray(g8.transpose(2, 0, 1)).reshape(128, NQ * 128)

    tgt = np.ascontiguousarray(ctx8[tid].T)  # [128, NCOL] fp8
    d0f = user_emb[uid] - ctx32[tid]  # [NCOL, 128] f32
    d0f[~valid] = 0.0
    d0T = np.ascontiguousarray(d0f.T).astype(BF16NP)

    in_map = {
        "prefT": prefT,
        "prefQ": prefQ,
        "tgt": tgt,
        "d0": d0T,
        "maskb": np.ascontiguousarray(mask.astype(BF16NP)),
        "ones8": np.ones((128, 128), FP8NP),
        "onesc": np.ones((128, 1), BF16NP),
    }
    return in_map, colsamp, colt, valid


def kernel(user_emb, item_emb, user_ids, pos_ids, neg_ids, pref_ids, n_prefs,
           _trace=False):
    user_emb = np.ascontiguousarray(np.asarray(user_emb, np.float32))
    item_emb = np.asarray(item_emb, np.float32)
    user_ids = np.asarray(user_ids).astype(np.int64)
    pos_ids = np.asarray(pos_ids).astype(np.int64)
    neg_ids = np.asarray(neg_ids).astype(np.int64)
    pref_ids = np.asarray(pref_ids).astype(np.int64)
    n_prefs = np.asarray(n_prefs, np.float32)

    ctx32 = np.concatenate([item_emb, np.zeros((1, D), np.float32)], axis=0)
    ctx8 = ctx32.astype(FP8NP)

    v = n_prefs.astype(np.int64) + 1  # valid slot counts

    pk, nc = _get(v)

    in_maps = []
    unscr = []
    for core in range(N_CORES):
        im, colsamp, colt, valid = prep_core(
            pk, core, ctx32, ctx8, user_emb, user_ids, pos_ids, neg_ids,
            pref_ids, v
        )
        in_maps.append(im)
        unscr.append((colsamp, colt, valid))

    res = run_bass_kernel_spmd(
        nc, in_maps, core_ids=list(range(N_CORES)), trace=_trace
    )

    out = np.empty((2, B), dtype=np.float32)
    for core in range(N_CORES):
        raw = np.asarray(res.results[core]["out"]).reshape(pk.NCOL)
        srow = np.asarray(res.results[core]["sout"]).reshape(pk.NCOL)
        colsamp, colt, valid = unscr[core]
        r = raw[valid] / np.square(srow[valid])
        out[colt[valid], colsamp[valid]] = r
    if _trace:
        return out, res
    return out


# revision 18
# speedup vs baseline: 1.1998x; 1.0686x over previous
"""AttCML distributed Bass kernel for 8 TRN2 NeuronCores — exact-packing v2.

Data-parallel over batch; both attention contractions on the PE array.

Key design points (v2, vs. the geometry-pool baseline):

  - input-adaptive packing: the Bass program is built inside kernel() after
    seeing n_prefs.  Samples are sorted by v = n_prefs+1 descending and dealt
    round-robin to the 8 cores by rank, so all cores share one program whose
    per-rank segment size is the max v across cores at that rank (+0.04%).
    Segments are first-fit-decreasing bin-packed into 128-slot "quads"
    (26.1 slots/sample vs 37.2 for fixed pools), then quads are FFD-packed by
    m = 2*samples into 128-column PSUM groups; 4 groups = one superblock
    ([128,512] = one PSUM bank).
  - per-column additive mask tensor [128, NCOL] shipped from host replaces
    the per-geometry block masks AND the pad-count correction: rows beyond a
    sample's v get -30 so fp8 exp underflows to exact 0.
  - all DMA-resident tiles are distinct SBUF allocations (no buffer reuse →
    no mid-stream DMA stalls); transfers are issued 4 superblocks ahead on
    the two HWDGE rings (sync: tgt+prefT+mask, scalar: prefQ+d0).
  - softmax denominator: ones-matmul broadcast of S + dead-col +1 via a
    [1,NCOL] row matmul; 1/S via reciprocal_approx_fast (custom DVE, ~5x
    faster than InstReciprocal and more accurate than the old bf16 path).
  - distances: rm = rps*(1/S) (DVE), qv = rm + d0 (DVE), square (ACT),
    ones-column matmul over d, [1,512] row copied and DMAed per superblock
    on the sync ring (queued behind the remaining input stream, so it never
    delays the tail).

Pad slots inside a segment are masked (-30); zero-id slots below v are real
zero rows contributing exp(0)=1 — exact reference semantics.
"""

import numpy as np
from contextlib import ExitStack

try:
    import concourse  # noqa: F401
except ImportError:  # pragma: no cover
    import sys

    for _p in ("/opt/trn_rl_repo", "/root/.axon_site/_ro/trn_rl_repo"):
        if _p not in sys.path:
            sys.path.insert(0, _p)

import ml_dtypes
import concourse.bacc as bacc
import concourse.bass as bass
import concourse.tile as tile
from concourse import mybir
from concourse.bass_utils import run_bass_kernel_spmd

F32 = mybir.dt.float32
BF16 = mybir.dt.bfloat16
FP8 = mybir.dt.float8e3  # e3m4
ALU = mybir.AluOpType
ACTF = mybir.ActivationFunctionType

FP8NP = ml_dtypes.float8_e3m4
BF16NP = ml_dtypes.bfloat16

D = 128
P = 50
N_CORES = 8
B = 16384
NRANK = B // N_CORES
MASKVAL = -30.0
LOOK = 4  # superblock DMA lookahead


class Packing:
    """Compile-time packing derived from the (global) v = n_prefs+1 array."""

    def __init__(self, v):
        order = np.argsort(-v, kind="stable")
        vs = v[order].reshape(NRANK, N_CORES)
        seg = vs.max(axis=1).astype(np.int64)  # descending

        # FFD: ranks into 128-slot quads
        quads = []  # [remaining, [rank, ...]]
        for i in range(NRANK):
            s = int(seg[i])
            for q in quads:
                if q[0] >= s:
                    q[0] -= s
                    q[1].append(i)
                    break
            else:
                quads.append([128 - s, [i]])

        # FFD: quads into 128-col groups by m = 2*samples
        ms = sorted(((2 * len(q[1]), qi) for qi, q in enumerate(quads)),
                    reverse=True)
        groups = []  # [remaining_cols, [quad_id, ...]]
        for m, qi in ms:
            for g in groups:
                if g[0] >= m:
                    g[0] -= m
                    g[1].append(qi)
                    break
            else:
                groups.append([128 - m, [qi]])

        NG = len(groups)
        # renumber quads in group order -> contiguous slot array per group
        self.order = order
        self.seg = seg
        self.NG = NG
        self.NCOL = NG * 128
        # per group: list of (new_quad_idx, colbase, [(rank, rowbase, seg)])
        self.groups = []
        rank_info = np.zeros((NRANK, 3), np.int64)  # quad, rowbase, colbase
        qn = 0
        for g, (_rem, qids) in enumerate(groups):
            glist = []
            col = g * 128
            for qi in qids:
                rows = []
                rb = 0
                for k, r in enumerate(quads[qi][1]):
                    rank_info[r] = (qn, rb, col + 2 * k)
                    rows.append((r, rb, int(seg[r])))
                    rb += int(seg[r])
                glist.append((qn, col, 2 * len(rows), rows))
                col += 2 * len(rows)
                qn += 1
            self.groups.append(glist)
        self.NQ = qn
        self.rank_info = rank_info

        # superblocks: small ramp-in and ramp-out, 4-group body
        sizes = [1, 1, 2]
        rem = NG - 4 - 4  # reserve [2,1,1] tail
        while rem >= 4:
            sizes.append(4)
            rem -= 4
        if rem:
            sizes.append(rem)
        sizes += [2, 1, 1]
        assert sum(sizes) == NG
        self.SBS = []
        g0 = 0
        for sz in sizes:
            self.SBS.append((g0, sz))
            g0 += sz

        # per-sb quad ranges (quads are contiguous per group, groups per sb)
        self.sb_q = []
        for g0, ng in self.SBS:
            q0 = self.groups[g0][0][0]
            qlast = self.groups[g0 + ng - 1][-1]
            self.sb_q.append((q0, qlast[0] + 1))

        # fused fp8 stream on the sync ring: per sb [tgt | mask | prefT]
        self.fa_off = []
        off = 0
        for sb, (g0, ng) in enumerate(self.SBS):
            q0, q1 = self.sb_q[sb]
            wid = ng * 128
            qwid = (q1 - q0) * 128
            self.fa_off.append((off, wid, qwid))
            off += 2 * wid + qwid
        self.FA = off

        # vectorization helpers for host prep
        lens = seg  # per rank
        self.tot = int(lens.sum())
        starts = rank_info[:, 0] * 128 + rank_info[:, 1]  # flat slot index
        rep_rank = np.repeat(np.arange(NRANK), lens)
        off_in_seg = np.arange(self.tot) - np.repeat(
            np.cumsum(lens) - lens, lens
        )
        self.flat_slot = np.repeat(starts, lens) + off_in_seg
        self.rep_rank = rep_rank
        self.off_in_seg = off_in_seg


def build_bass(pk: Packing):
    NQ, NCOL = pk.NQ, pk.NCOL
    SBS = pk.SBS
    NSB = len(SBS)

    nc = bacc.Bacc(
        "TRN2",
        target_bir_lowering=False,
        debug=False,
        enable_asserts=False,
        num_devices=N_CORES,
    )

    fa_in = nc.declare_dram_parameter("fa", [128, pk.FA], FP8, isOutput=False)
    prefQ_in = nc.declare_dram_parameter("prefQ", [128, NQ * 128], FP8, isOutput=False)
    d0_in = nc.declare_dram_parameter("d0", [128, NCOL], BF16, isOutput=False)
    ones8_in = nc.declare_dram_parameter("ones8", [128, 128], FP8, isOutput=False)
    onesc_in = nc.declare_dram_parameter("onesc", [128, 1], BF16, isOutput=False)
    out_d = nc.declare_dram_parameter("out", [1, NCOL], F32, isOutput=True)
    s_d = nc.declare_dram_parameter("sout", [1, NCOL], F32, isOutput=True)

    with tile.TileContext(nc) as tc, ExitStack() as ctx:
        ctx.enter_context(
            nc.allow_low_precision(reason="fp8/bf16 pipeline validated vs reference")
        )
        consts = ctx.enter_context(tc.tile_pool(name="consts", bufs=1))
        res_pool = ctx.enter_context(tc.tile_pool(name="res", bufs=1))
        sm_pool = ctx.enter_context(tc.tile_pool(name="sm", bufs=3))
        row_pool = ctx.enter_context(tc.tile_pool(name="row", bufs=2))
        w_ps = ctx.enter_context(
            tc.tile_pool(name="wps", bufs=2, space=bass.MemorySpace.PSUM)
        )
        s_ps = ctx.enter_context(
            tc.tile_pool(name="sps", bufs=2, space=bass.MemorySpace.PSUM)
        )
        r_ps = ctx.enter_context(
            tc.tile_pool(name="rps", bufs=2, space=bass.MemorySpace.PSUM)
        )
        o_ps = ctx.enter_context(
            tc.tile_pool(name="ops", bufs=2, space=bass.MemorySpace.PSUM)
        )

        # consts on the scalar ring (small, needed from the first superblock)
        ones8 = consts.tile([128, 128], FP8)
        nc.scalar.dma_start(ones8[:], ones8_in[:])
        onesc = consts.tile([128, 1], BF16)
        nc.scalar.dma_start(onesc[:], onesc_in[:])

        sbT = [None] * NSB
        sbC = [None] * NSB

        def issue_T(sb):
            # one fused transfer: [tgt | mask | prefT] fp8
            off, wid, qwid = pk.fa_off[sb]
            fa = res_pool.tile([128, 2 * wid + qwid], FP8, tag=f"fa{sb}", name="fa")
            nc.sync.dma_start(fa[:], fa_in[:, off : off + 2 * wid + qwid])
            sbT[sb] = (fa[:, :wid], fa[:, wid : 2 * wid], fa[:, 2 * wid :])

        def issue_C(sb):
            g0, ng = SBS[sb]
            q0, q1 = pk.sb_q[sb]
            wid = ng * 128
            pQ = res_pool.tile([128, (q1 - q0) * 128], FP8, tag=f"pQ{sb}", name="pQ")
            nc.scalar.dma_start(pQ[:], prefQ_in[:, q0 * 128 : q1 * 128])
            d0 = res_pool.tile([128, wid], BF16, tag=f"d0{sb}", name="d0")
            nc.scalar.dma_start(d0[:], d0_in[:, g0 * 128 : g0 * 128 + wid])
            sbC[sb] = (pQ, d0)

        def quads_mm(sb, dst, src_pref, mov):
            g0, ng = SBS[sb]
            q0, _ = pk.sb_q[sb]
            c0 = g0 * 128
            for g in range(g0, g0 + ng):
                for qn, col, m, _rows in pk.groups[g]:
                    lc = col - c0
                    nc.tensor.matmul(
                        dst[:, lc : lc + m],
                        src_pref[:, (qn - q0) * 128 : (qn - q0 + 1) * 128],
                        mov[:, lc : lc + m],
                    )

        from collections import deque

        pend_q2 = deque()
        pend_fin = deque()

        def emit_dist():
            # two iterations behind exp: PE/ACT never wait on fresh q2
            if not pend_q2:
                return
            q2, c0, wid = pend_q2.popleft()
            ops = o_ps.tile([1, 512], F32, tag="o", name="ops")
            nc.tensor.matmul(ops[:, :wid], onesc[:], q2[:, :wid])
            orow = row_pool.tile([1, 512], F32, tag="or", name="orow")
            nc.scalar.copy(orow[:, :wid], ops[:, :wid])
            nc.gpsimd.dma_start(out_d[:, c0 : c0 + wid], orow[:, :wid])

        def emit_fin():
            # one iteration behind exp: DVE reads only settled PSUM
            if not pend_fin:
                return
            sb, sps, rps = pend_fin.popleft()
            g0, ng = SBS[sb]
            pQ, d0 = sbC[sb]
            c0 = g0 * 128
            wid = ng * 128
            srow = row_pool.tile([1, 512], F32, tag="sr", name="srow")
            nc.vector.tensor_copy(out=srow[:, :wid], in_=sps[0:1, :wid])
            nc.gpsimd.dma_start(s_d[:, c0 : c0 + wid], srow[:, :wid])
            m1 = sm_pool.tile([128, 512], BF16, tag="m1", name="m1")
            nc.vector.tensor_tensor(
                out=m1[:, :wid], in0=sps[:, :wid], in1=d0[:, :wid], op=ALU.mult
            )
            qv = sm_pool.tile([128, 512], BF16, tag="qv", name="qv")
            nc.vector.tensor_tensor(
                out=qv[:, :wid], in0=rps[:, :wid], in1=m1[:, :wid], op=ALU.add
            )
            q2 = sm_pool.tile([128, 512], BF16, tag="q2", name="q2")
            nc.vector.tensor_tensor(
                out=q2[:, :wid], in0=qv[:, :wid], in1=qv[:, :wid], op=ALU.mult
            )
            pend_q2.append((q2, c0, wid))

        def stage_a(sb):
            tg, mk, pT = sbT[sb]
            wps = w_ps.tile([128, 512], F32, tag="w", name="wps")
            quads_mm(sb, wps, pT, tg)
            return wps

        def head(sb, wps):
            # mask+exp+S+pool for sb (qv/q2 and dist emitted in later its)
            g0, ng = SBS[sb]
            tg, mk, pT = sbT[sb]
            pQ, d0 = sbC[sb]
            wid = ng * 128

            wm = sm_pool.tile([128, 512], BF16, tag="wm", name="wm")
            nc.vector.tensor_tensor(
                out=wm[:, :wid], in0=wps[:, :wid], in1=mk[:, :wid], op=ALU.add
            )
            ee = sm_pool.tile([128, 512], FP8, tag="ee", name="ee")
            nc.scalar.activation(ee[:, :wid], wm[:, :wid], ACTF.Exp)

            sps = s_ps.tile([128, 512], F32, tag="s", name="sps")
            nc.tensor.matmul(sps[:, :wid], ones8[:], ee[:, :wid])
            rps = r_ps.tile([128, 512], F32, tag="r", name="rps")
            quads_mm(sb, rps, pQ, ee)
            pend_fin.append((sb, sps, rps))

        LOOK_T, LOOK_C = 5, 4
        for sb in range(min(LOOK_T, NSB)):
            issue_T(sb)
            if sb < LOOK_C:
                issue_C(sb)
        wcur = stage_a(0)
        for sb in range(NSB):
            if sb + LOOK_T < NSB:
                issue_T(sb + LOOK_T)
            if sb + LOOK_C < NSB:
                issue_C(sb + LOOK_C)
            emit_dist()
            wnext = stage_a(sb + 1) if sb + 1 < NSB else None
            head(sb, wcur)
            if len(pend_fin) > 1:
                emit_fin()
            wcur = wnext
        emit_dist()
        emit_fin()
        emit_dist()

    nc.compile()
    return nc


_CACHE = {}


def _get(v):
    key = v.tobytes()
    if _CACHE.get("key") != key:
        pk = Packing(v)
        nc = build_bass(pk)
        _CACHE.update(key=key, pk=pk, nc=nc)
    return _CACHE["pk"], _CACHE["nc"]


def prep_core(pk, core, ctx32, ctx8, user_emb, user_ids, pos_ids, neg_ids,
              pref_ids, v):
    """Build one core's input map + unscramble info."""
    NQ, NCOL = pk.NQ, pk.NCOL
    ZERO = ctx8.shape[0] - 1

    samples = pk.order[np.arange(NRANK) * N_CORES + core]  # per rank
    vc = v[samples]  # <= seg per construction

    # slot id array [NQ*128]
    sid = np.full(NQ * 128, ZERO, np.int64)
    keep = pk.off_in_seg < vc[pk.rep_rank]
    fs = pk.flat_slot[keep]
    sid[fs] = pref_ids[samples[pk.rep_rank[keep]], pk.off_in_seg[keep]]
    sid = sid.reshape(NQ, 128)

    # mask [128, NCOL]: 0 for (slot rows < vc) of each sample's two columns
    mask = np.full((128, NCOL), MASKVAL, np.float32)
    rows = (pk.rank_info[pk.rep_rank[keep], 1] + pk.off_in_seg[keep])
    colp = pk.rank_info[pk.rep_rank[keep], 2]
    mask[rows, colp] = 0.0
    mask[rows, colp + 1] = 0.0

    # per-column targets
    colsamp = np.full(NCOL, -1, np.int64)
    colt = np.zeros(NCOL, np.int64)
    tid = np.full(NCOL, ZERO, np.int64)
    uid = np.zeros(NCOL, np.int64)
    valid = np.zeros(NCOL, bool)
    cp = pk.rank_info[:, 2]
    for t, t_ids in ((0, pos_ids), (1, neg_ids)):
        cc = cp + t
        colsamp[cc] = samples
        colt[cc] = t
        tid[cc] = t_ids[samples]
        uid[cc] = user_ids[samples]
        valid[cc] = True

    g8 = ctx8[sid]  # [NQ, 128, 128]
    prefQ = np.ascontiguousarray(g8.transpose(1, 0, 2)).reshape(128, NQ * 128)
    prefT = np.ascontiguousarray(g8.transpose(2, 0, 1)).reshape(128, NQ * 128)

    tgt = ctx8[tid].T  # [128, NCOL] fp8
    d0f = user_emb[uid] - ctx32[tid]  # [NCOL, 128] f32
    d0f[~valid] = 0.0
    d0T = np.ascontiguousarray(d0f.T).astype(BF16NP)

    # fused sync-ring stream: per sb [tgt | mask | prefT], all fp8
    mask8 = mask.astype(FP8NP)
    tgt8 = tgt.astype(FP8NP)
    fa = np.empty((128, pk.FA), FP8NP)
    for sb, (g0, ng) in enumerate(pk.SBS):
        off, wid, qwid = pk.fa_off[sb]
        q0, q1 = pk.sb_q[sb]
        c0 = g0 * 128
        fa[:, off : off + wid] = tgt8[:, c0 : c0 + wid]
        fa[:, off + wid : off + 2 * wid] = mask8[:, c0 : c0 + wid]
        fa[:, off + 2 * wid : off + 2 * wid + qwid] = prefT[:, q0 * 128 : q1 * 128]

    in_map = {
        "fa": fa,
        "prefQ": prefQ,
        "d0": d0T,
        "ones8": np.ones((128, 128), FP8NP),
        "onesc": np.ones((128, 1), BF16NP),
    }
    return in_map, colsamp, colt, valid


def kernel(user_emb, item_emb, user_ids, pos_ids, neg_ids, pref_ids, n_prefs,
           _trace=False):
    user_emb = np.ascontiguousarray(np.asarray(user_emb, np.float32))
    item_emb = np.asarray(item_emb, np.float32)
    user_ids = np.asarray(user_ids).astype(np.int64)
    pos_ids = np.asarray(pos_ids).astype(np.int64)
    neg_ids = np.asarray(neg_ids).astype(np.int64)
    pref_ids = np.asarray(pref_ids).astype(np.int64)
    n_prefs = np.asarray(n_prefs, np.float32)

    ctx32 = np.concatenate([item_emb, np.zeros((1, D), np.float32)], axis=0)
    ctx8 = ctx32.astype(FP8NP)

    v = n_prefs.astype(np.int64) + 1  # valid slot counts

    pk, nc = _get(v)

    in_maps = []
    unscr = []
    for core in range(N_CORES):
        im, colsamp, colt, valid = prep_core(
            pk, core, ctx32, ctx8, user_emb, user_ids, pos_ids, neg_ids,
            pref_ids, v
        )
        in_maps.append(im)
        unscr.append((colsamp, colt, valid))

    res = run_bass_kernel_spmd(
        nc, in_maps, core_ids=list(range(N_CORES)), trace=_trace
    )

    out = np.empty((2, B), dtype=np.float32)
    for core in range(N_CORES):
        raw = np.asarray(res.results[core]["out"]).reshape(pk.NCOL)
        srow = np.asarray(res.results[core]["sout"]).reshape(pk.NCOL)
        colsamp, colt, valid = unscr[core]
        r = raw[valid] / np.square(srow[valid])
        out[colt[valid], colsamp[valid]] = r
    if _trace:
        return out, res
    return out
